# revision 10
# baseline (speedup 1.0000x reference)
"""AttentionFusion kernel for 8 TRN2 NeuronCores — pure data parallel.

Pipeline (feature-major: features on SBUF partitions, batch on free dim):
  1. DMA inputs batch-major, PE-transpose to feature-major
  2. Per-modality projections (PE matmuls, bias via ACT copy)
  3. QKV projections (PE), softmax-scale folded into Wq on host
  4. Attention over 5 tokens: DVE elementwise q_i*k_j products,
     PE head-mask matmul reduces d (32) and broadcasts back to (h,d),
     ACT exp, DVE tree-sum for Z, ACT reciprocal, DVE normalize,
     DVE attn*v products, PE out-projection with j-accumulation in PSUM
     (+ residual via identity matmul, out_b via ACT bias)
  5. LayerNorm: mean/var via all-1/128 matmul (broadcast built in),
     ACT rsqrt, DVE apply; ln_g/ln_b folded into fusion weights on host
  6. Fusion matmul emits batch-major output directly (y_norm^T as lhsT)
"""

import os
import sys
from contextlib import ExitStack

import numpy as np

sys.path.insert(0, "/opt/trn_rl_repo")

import ml_dtypes

BF16 = ml_dtypes.bfloat16

B_TOTAL = 65536
N_CORES = 8
B_CORE = B_TOTAL // N_CORES  # 8192
B_CHUNK = 512
N_CHUNK = B_CORE // B_CHUNK  # 16
ATT = 128
H = 4
DH = 32
NMOD = 5
TOTAL = 992
EPS = 1e-5

MODS = [
    ("category_emb", 128),
    ("menu_emb", 256),
    ("diner_name_emb", 64),
    ("price_emb", 32),
    ("review_text_emb", 512),
]


def _build_consts(inp):
    """Host-side constant preparation (weights pre-transposed, bf16)."""
    f = lambda x: np.asarray(x, np.float32)

    def ktile_pack(wT):  # [K, 128] -> [128, ceil(K/128)*128] (k-major columns)
        K = wT.shape[0]
        nk = (K + 127) // 128
        out = np.zeros((128, nk * 128), np.float32)
        for kt in range(nk):
            w = min(128, K - kt * 128)
            out[:w, kt * 128 : kt * 128 + 128] = wT[kt * 128 : kt * 128 + w, :]
        return out

    c = {}
    scale = 1.0 / np.sqrt(DH)
    in_w, in_b = f(inp["in_w"]), f(inp["in_b"])
    Wq, Wk, Wv = in_w[0:128], in_w[128:256], in_w[256:384]
    c["wt_c"] = f(inp["Wc"]).T.astype(BF16)  # [128,128]
    c["wt_m"] = ktile_pack(f(inp["Wm"]).T).astype(BF16)  # [128,256]
    c["wt_d"] = f(inp["Wd"]).T.astype(BF16)  # [64,128]
    c["wt_p"] = f(inp["Wp"]).T.astype(BF16)  # [32,128]
    c["wt_r"] = ktile_pack(f(inp["Wr"]).T).astype(BF16)  # [128,512]
    c["wt_q"] = (Wq * scale).T.astype(BF16)
    c["wt_k"] = Wk.T.astype(BF16)
    c["wt_v"] = Wv.T.astype(BF16)
    c["wt_o"] = f(inp["out_w"]).T.astype(BF16)
    # fusion: fold ln_g/ln_b
    fus_w, fus_b = f(inp["fus_w"]), f(inp["fus_b"])
    ln_g, ln_b = f(inp["ln_g"]), f(inp["ln_b"])
    lg = np.tile(ln_g, NMOD)
    lb = np.tile(ln_b, NMOD)
    fw = fus_w * lg[None, :]  # [992, 640]
    fb = fus_b + fus_w @ lb  # [992]
    fwT = fw.T  # [640, 992]
    c["fus_wt"] = (
        fwT.reshape(NMOD, 128, TOTAL).transpose(1, 0, 2).reshape(128, NMOD * TOTAL)
    ).astype(BF16)
    c["fus_b"] = fb.reshape(1, TOTAL).astype(BF16)
    # masks / helpers
    k_idx = np.arange(128)
    c["mask_h"] = (k_idx[:, None] // DH == k_idx[None, :] // DH).astype(BF16)
    c["id_f32"] = np.eye(128, dtype=np.float32)
    c["id_bf"] = np.eye(128).astype(BF16)
    c["mean_m"] = np.full((128, 128), 1.0 / 128.0).astype(BF16)
    c["ones_1"] = np.ones((1, 128)).astype(BF16)
    biases = np.zeros((128, 10), np.float32)
    for m, nm in enumerate(["bc", "bm", "bd", "bp", "br"]):
        biases[:, m] = f(inp[nm])
    biases[:, 5] = in_b[0:128] * scale
    biases[:, 6] = in_b[128:256]
    biases[:, 7] = in_b[256:384]
    biases[:, 8] = f(inp["out_b"])
    biases[:, 9] = EPS
    c["biases"] = biases
    return c


def _build_module():
    import concourse.bass as bass
    import concourse.tile as tile
    from concourse import bacc, mybir

    fp32 = mybir.dt.float32
    bf16 = mybir.dt.bfloat16
    AF = mybir.ActivationFunctionType

    nc = bacc.Bacc("TRN2", target_bir_lowering=False, debug=False,
                   num_devices=N_CORES)

    dram = {}
    for name, d in MODS:
        dram[name] = nc.dram_tensor(name, [B_CORE, d], fp32, kind="ExternalInput")
    out_d = nc.dram_tensor("out", [B_CORE, TOTAL], fp32, kind="ExternalOutput")
    cshapes = {
        "wt_c": ([128, 128], bf16), "wt_m": ([128, 256], bf16),
        "wt_d": ([64, 128], bf16), "wt_p": ([32, 128], bf16),
        "wt_r": ([128, 512], bf16), "wt_q": ([128, 128], bf16),
        "wt_k": ([128, 128], bf16), "wt_v": ([128, 128], bf16),
        "wt_o": ([128, 128], bf16),
        "fus_wt": ([128, NMOD * TOTAL], bf16), "fus_b": ([1, TOTAL], bf16),
        "mask_h": ([128, 128], bf16), "id_f32": ([128, 128], fp32),
        "id_bf": ([128, 128], bf16), "mean_m": ([128, 128], bf16),
        "ones_1": ([1, 128], bf16), "biases": ([128, 10], fp32),
    }
    for name, (shape, dt) in cshapes.items():
        dram[name] = nc.dram_tensor(name, shape, dt, kind="ExternalInput")

    with tile.TileContext(nc) as tc, ExitStack() as ctx:
        cpool = ctx.enter_context(tc.tile_pool(name="consts", bufs=1))
        natp = ctx.enter_context(tc.tile_pool(name="nat", bufs=3))
        embp = ctx.enter_context(tc.tile_pool(name="embT", bufs=2))
        medp = ctx.enter_context(tc.tile_pool(name="med", bufs=1))
        bigp = ctx.enter_context(tc.tile_pool(name="big", bufs=2))
        big2 = ctx.enter_context(tc.tile_pool(name="big2", bufs=1))
        outp = ctx.enter_context(tc.tile_pool(name="outs", bufs=2))
        pst = ctx.enter_context(tc.tile_pool(name="pst", bufs=2, space="PSUM"))
        ps1 = ctx.enter_context(tc.tile_pool(name="ps1", bufs=3, space="PSUM"))
        psf = ctx.enter_context(tc.tile_pool(name="psf", bufs=1, space="PSUM"))

        # ---- load constants into SBUF once ----
        cs = {}
        for name, (shape, dt) in cshapes.items():
            t = cpool.tile(shape, dt, tag=name, name=f"c_{name}")
            nc.sync.dma_start(t[:], dram[name][:])
            cs[name] = t
        wt_by_mod = [cs["wt_c"], cs["wt_m"], cs["wt_d"], cs["wt_p"], cs["wt_r"]]

        def chunk_body(c0):
            # ---------- input load + transpose ----------
            embT = []
            for m, (name, d) in enumerate(MODS):
                nk = (d + 127) // 128
                et = embp.tile([min(d, 128), nk * B_CHUNK], bf16, tag=f"embT{m}")
                embT.append(et)
                for g in range(4):
                    nat = natp.tile([128, d], fp32, tag=f"nat{m}")
                    nc.sync.dma_start(
                        nat[:], dram[name][bass.ds(c0 + g * 128, 128), :])
                    for kt in range(nk):
                        w = min(128, d - kt * 128)
                        tp = pst.tile([128, 128], fp32, tag="tp")
                        nc.tensor.transpose(
                            tp[0:w, :], nat[:, kt * 128 : kt * 128 + w],
                            cs["id_f32"][:])
                        nc.vector.tensor_copy(
                            et[0:w, kt * B_CHUNK + g * 128 :
                               kt * B_CHUNK + (g + 1) * 128],
                            tp[0:w, :])

            # ---------- projections ----------
            x_all = medp.tile([128, NMOD * B_CHUNK], bf16, tag="x")
            for m, (name, d) in enumerate(MODS):
                nk = (d + 127) // 128
                px = ps1.tile([128, B_CHUNK], fp32, tag="ps1")
                for kt in range(nk):
                    w = min(128, d - kt * 128)
                    nc.tensor.matmul(
                        px[:], wt_by_mod[m][0:w, kt * 128 : (kt + 1) * 128],
                        embT[m][0:w, kt * B_CHUNK : (kt + 1) * B_CHUNK],
                        start=(kt == 0), stop=(kt == nk - 1))
                nc.scalar.activation(
                    x_all[:, m * B_CHUNK : (m + 1) * B_CHUNK], px[:],
                    AF.Identity, bias=cs["biases"][:, m : m + 1])

            # ---------- qkv ----------
            qkv = {}
            for nm, wt, bcol in (("q", "wt_q", 5), ("k", "wt_k", 6),
                                 ("v", "wt_v", 7)):
                qkv[nm] = medp.tile([128, NMOD * B_CHUNK], bf16, tag=nm,
                                    name=f"qkv_{nm}")
            for m in range(NMOD):
                xs = x_all[:, m * B_CHUNK : (m + 1) * B_CHUNK]
                for nm, wt, bcol in (("q", "wt_q", 5), ("k", "wt_k", 6),
                                     ("v", "wt_v", 7)):
                    pq = ps1.tile([128, B_CHUNK], fp32, tag="ps1")
                    nc.tensor.matmul(pq[:], cs[wt][:], xs)
                    nc.scalar.activation(
                        qkv[nm][:, m * B_CHUNK : (m + 1) * B_CHUNK], pq[:],
                        AF.Identity, bias=cs["biases"][:, bcol : bcol + 1])

            # ---------- scores products: prod[p,(i,j,b)] = q_i * k_j ----------
            prod = bigp.tile([128, 25 * B_CHUNK], bf16, tag="big")
            q4 = (qkv["q"][:].rearrange("p (i b) -> p i b", i=NMOD)
                  .unsqueeze(2).to_broadcast((128, NMOD, NMOD, B_CHUNK)))
            k4 = (qkv["k"][:].rearrange("p (j b) -> p j b", j=NMOD)
                  .unsqueeze(1).to_broadcast((128, NMOD, NMOD, B_CHUNK)))
            p4 = prod[:].rearrange("p (i j b) -> p i j b", i=NMOD, j=NMOD)
            nc.vector.tensor_mul(p4, q4, k4)

            # ---------- head-reduce+broadcast, exp ----------
            e_all = big2.tile([128, 25 * B_CHUNK], bf16, tag="big2")
            for ij in range(25):
                sb = ps1.tile([128, B_CHUNK], fp32, tag="ps1")
                nc.tensor.matmul(
                    sb[:], cs["mask_h"][:],
                    prod[:, ij * B_CHUNK : (ij + 1) * B_CHUNK])
                nc.scalar.activation(
                    e_all[:, ij * B_CHUNK : (ij + 1) * B_CHUNK], sb[:], AF.Exp)

            # ---------- Z = sum_j e ----------
            e4 = e_all[:].rearrange("p (i j b) -> p i j b", i=NMOD, j=NMOD)
            zt = medp.tile([128, NMOD * 2 * B_CHUNK], bf16, tag="zt")
            zt4 = zt[:].rearrange("p (i j b) -> p i j b", i=NMOD, j=2)
            nc.vector.tensor_add(zt4[:, :, 0:1, :], e4[:, :, 0:1, :],
                                 e4[:, :, 1:2, :])
            nc.vector.tensor_add(zt4[:, :, 1:2, :], e4[:, :, 2:3, :],
                                 e4[:, :, 3:4, :])
            z_all = medp.tile([128, NMOD * B_CHUNK], bf16, tag="z")
            z4 = (z_all[:].rearrange("p (i b) -> p i b", i=NMOD).unsqueeze(2))
            nc.vector.tensor_add(z4, zt4[:, :, 0:1, :], zt4[:, :, 1:2, :])
            nc.vector.tensor_add(z4, z4, e4[:, :, 4:5, :])

            # recip via exp(-ln(Z)) — Reciprocal ACT func is blocked
            lnz = medp.tile([128, NMOD * B_CHUNK], bf16, tag="lnz")
            nc.scalar.activation(lnz[:], z_all[:], AF.Ln)
            rz = medp.tile([128, NMOD * B_CHUNK], bf16, tag="rz")
            nc.scalar.activation(rz[:], lnz[:], AF.Exp, scale=-1.0)

            # ---------- attn = e * rz (bcast over j), P = attn * v ----------
            attn = bigp.tile([128, 25 * B_CHUNK], bf16, tag="big")
            a4 = attn[:].rearrange("p (i j b) -> p i j b", i=NMOD, j=NMOD)
            r4 = (rz[:].rearrange("p (i b) -> p i b", i=NMOD)
                  .unsqueeze(2).to_broadcast((128, NMOD, NMOD, B_CHUNK)))
            nc.vector.tensor_mul(a4, e4, r4)
            pw = big2.tile([128, 25 * B_CHUNK], bf16, tag="big2")
            w4 = pw[:].rearrange("p (i j b) -> p i j b", i=NMOD, j=NMOD)
            v4 = (qkv["v"][:].rearrange("p (j b) -> p j b", j=NMOD)
                  .unsqueeze(1).to_broadcast((128, NMOD, NMOD, B_CHUNK)))
            nc.vector.tensor_mul(w4, a4, v4)

            # ---------- out-proj (+residual) ----------
            y_all = medp.tile([128, NMOD * B_CHUNK], bf16, tag="y")
            for i in range(NMOD):
                py = ps1.tile([128, B_CHUNK], fp32, tag="ps1")
                for j in range(NMOD):
                    nc.tensor.matmul(
                        py[:], cs["wt_o"][:],
                        pw[:, (i * NMOD + j) * B_CHUNK :
                           (i * NMOD + j + 1) * B_CHUNK],
                        start=(j == 0), stop=False)
                nc.tensor.matmul(py[:], cs["id_bf"][:],
                                 x_all[:, i * B_CHUNK : (i + 1) * B_CHUNK],
                                 start=False, stop=True)
                nc.scalar.activation(
                    y_all[:, i * B_CHUNK : (i + 1) * B_CHUNK], py[:],
                    AF.Identity, bias=cs["biases"][:, 8:9])

            # ---------- LayerNorm ----------
            yc = medp.tile([128, NMOD * B_CHUNK], bf16, tag="yc")
            for i in range(NMOD):
                pm = ps1.tile([128, B_CHUNK], fp32, tag="ps1")
                nc.tensor.matmul(pm[:], cs["mean_m"][:],
                                 y_all[:, i * B_CHUNK : (i + 1) * B_CHUNK])
                nc.vector.tensor_sub(
                    yc[:, i * B_CHUNK : (i + 1) * B_CHUNK],
                    y_all[:, i * B_CHUNK : (i + 1) * B_CHUNK], pm[:])
            ysq = medp.tile([128, NMOD * B_CHUNK], bf16, tag="ysq")
            nc.vector.tensor_mul(ysq[:], yc[:], yc[:])
            # rstd = exp(-0.5*ln(var+eps)) — Rsqrt ACT func is blocked
            lnv = medp.tile([128, NMOD * B_CHUNK], bf16, tag="lnv")
            for i in range(NMOD):
                pv = ps1.tile([128, B_CHUNK], fp32, tag="ps1")
                nc.tensor.matmul(pv[:], cs["mean_m"][:],
                                 ysq[:, i * B_CHUNK : (i + 1) * B_CHUNK])
                nc.scalar.activation(
                    lnv[:, i * B_CHUNK : (i + 1) * B_CHUNK], pv[:],
                    AF.Ln, bias=cs["biases"][:, 9:10])
            rstd = medp.tile([128, NMOD * B_CHUNK], bf16, tag="rstd")
            nc.scalar.activation(rstd[:], lnv[:], AF.Exp, scale=-0.5)
            ynorm = medp.tile([128, NMOD * B_CHUNK], bf16, tag="yn")
            nc.vector.tensor_mul(ynorm[:], yc[:], rstd[:])

            # ---------- fusion (emits batch-major) ----------
            for s in range(4):
                po = psf.tile([128, TOTAL], fp32, tag="psf")
                for i in range(NMOD):
                    lhsT = ynorm[:, i * B_CHUNK + s * 128 :
                                 i * B_CHUNK + (s + 1) * 128]
                    nc.tensor.matmul(po[:, 0:512], lhsT,
                                     cs["fus_wt"][:, i * TOTAL : i * TOTAL + 512],
                                     start=(i == 0), stop=False)
                    nc.tensor.matmul(po[:, 512:TOTAL], lhsT,
                                     cs["fus_wt"][:, i * TOTAL + 512 :
                                                   (i + 1) * TOTAL],
                                     start=(i == 0), stop=False)
                nc.tensor.matmul(po[:, 0:512], cs["ones_1"][:],
                                 cs["fus_b"][:, 0:512], start=False, stop=True)
                nc.tensor.matmul(po[:, 512:TOTAL], cs["ones_1"][:],
                                 cs["fus_b"][:, 512:TOTAL], start=False,
                                 stop=True)
                osb = outp.tile([128, TOTAL], fp32, tag="osb")
                if s % 2 == 0:
                    nc.scalar.activation(osb[:], po[:], AF.Copy)
                else:
                    nc.vector.tensor_copy(osb[:], po[:])
                nc.sync.dma_start(out_d[bass.ds(c0 + s * 128, 128), :], osb[:])

        with tc.For_i(0, B_CORE, B_CHUNK) as c0:
            chunk_body(c0)

    nc.compile()
    return nc


_NC_CACHE = None


def kernel(**inputs):
    global _NC_CACHE
    inputs = {k: np.asarray(v) for k, v in inputs.items()}
    consts = _build_consts(inputs)
    if _NC_CACHE is None:
        _NC_CACHE = _build_module()
    nc = _NC_CACHE
    from concourse.bass_utils import run_bass_kernel_spmd

    in_maps = []
    for core in range(N_CORES):
        sl = slice(core * B_CORE, (core + 1) * B_CORE)
        m = {name: np.ascontiguousarray(inputs[name][sl], dtype=np.float32)
             for name, _ in MODS}
        m.update(consts)
        in_maps.append(m)
    res = run_bass_kernel_spmd(nc, in_maps, core_ids=list(range(N_CORES)),
                               trace=os.environ.get("KTRACE", "0") == "1")
    if res.exec_time_ns is not None:
        print(f"HW exec time: {res.exec_time_ns} ns")
    out = np.concatenate([r["out"] for r in res.results], axis=0)
    return out.astype(np.float32)
